# revision 18
# baseline (speedup 1.0000x reference)
"""Cost-volume kernel for Trainium2 (Bass/Tile), SPMD over 8 NeuronCores.

volume[b, d, h, w] = mean_c left[b,c,h,w] * right[b,c,h,w-d],  0 for w < d.

Per core (one batch image b), per 8-row chunk (CH=8, 20 chunks):
  - Host pre-reverses right along w (rp[c,x] = R[c, 319-x]) and pre-scales
    left by 1/64 (the channel mean, exact power of two), both cast to bf16.
  - TensorE: per (hh, wb) one [32, <=80] matmul over w-blocks of BM=32
      G[q, f] = sum_c L[c, 32*wb+q] * rp[c, 288-32*wb+f]
             = vol_unscaled[d = q + f - 31, h, w = 32*wb + q]
    so each PSUM row is d-contiguous.  Columns with w2 = 31+32*wb-f < 0
    (the w < d zero triangle) are skipped (cols = min(80, 32*wb+32)); the
    host masks them.  Four hh's stack at 32-row offsets (matmul
    tile_position col in {0,32,64,96}) into [128, 320] PSUM tiles, 4
    col-regions per tile, 5 tiles per chunk.
  - DVE (tiles 0,2) / ACT (tiles 1,3,4) evict PSUM -> SBUF band
    [128, 1600] with the f32 -> bf16 cast.
  - ONE contiguous store DMA per chunk: band -> DRAM out[c] (3200B runs,
    full DMA bandwidth; no on-chip reshuffle at all).  The final chunk
    stores per-tile on SP/HWDGE so the drain tail is one tile deep.
Host: gather the 48 diagonals f = 31 - q + d from the band (pure
selection, numpy take_along_axis), mask w < d to exact 0, reshape to
[D, H, W] f32.

Why this shape (TimelineSim cost model, which tracked the measured
baseline within 4%):
  - DMA_ENGINES is the bottleneck resource: busy = loads 36.4us (13.1MB
    bf16 inputs at 360GB/s; bf16 is required - fp8 inputs put ~3.6% rms on
    the output vs the 2e-2 gate) + stores 22.8us.  The 80-wide band
    regions store 1.67x the output bytes but at the full contiguous rate;
    runs under 512B are charged 2x, so trimming the band below full rows
    always loses.  BM=32 balances store overhead ((BM+48)/48) against PE
    column time ((BM+48)/BM): PE ~50us < DMA ~59us.
  - A diagonal-extracting store would need a sub-row partition step on the
    DMA *read* side, which the BIR verifier rejects ("illegal partition
    step" - writes only); hence store-the-band + host selection.
  - Loads go on SP/HWDGE, the band store on Pool/SWDGE: a shared queue
    would serialize chunk c+1's loads behind chunk c's eviction semaphore
    (the store must wait for evictions), collapsing the pipeline.
"""

import sys

sys.path.insert(0, "/opt/trn_rl_repo")

import numpy as np

import concourse.tile as tile
from concourse import bacc, mybir

B, C, H, W, D = 8, 64, 160, 320, 48
MARGIN = 48
BM = 32                      # w-block size
BANDW = BM + MARGIN          # 80 band cols per region
NWB = W // BM                # 10 blocks
CH = 8                       # h rows per chunk
NK = 2 * NWB                 # 20 col-regions per chunk (2 sections x 10 wb)
PKW = NK * BANDW             # 1600 band cols
NTILE = 5                    # PSUM tiles per chunk (4 regions each)

_cache = {}


def _build(h_count=H, reps=1):
    bf16 = mybir.dt.bfloat16
    f32 = mybir.dt.float32
    assert h_count % CH == 0
    nchunk = h_count // CH

    nc = bacc.Bacc("TRN2", target_bir_lowering=False, debug=False)
    left = nc.dram_tensor("left", [C, h_count, W], bf16, kind="ExternalInput")
    right = nc.dram_tensor("right", [C, h_count, W], bf16, kind="ExternalInput")
    if reps != 1:
        # unused; forces a distinct HLO per reps so the jit/NEFF caches
        # cannot alias timing builds of different rep counts
        nc.dram_tensor("rep_tag", [1, 8 * reps], mybir.dt.float32,
                       kind="ExternalInput")
    out = nc.dram_tensor("out", [nchunk, 128, PKW], bf16, kind="ExternalOutput")

    with tile.TileContext(nc) as tc:
        with (
            tc.tile_pool(name="lt", bufs=6) as lt_pool,
            tc.tile_pool(name="rp", bufs=6) as rp_pool,
            tc.tile_pool(name="ps", bufs=8, space="PSUM") as ps_pool,
            tc.tile_pool(name="band", bufs=4) as band_pool,
        ):
            for ci in range(reps * nchunk):
                c = ci % nchunk
                h0 = c * CH
                lt = lt_pool.tile([C, CH, W], bf16)
                nc.sync.dma_start(lt[:], left[:, h0 : h0 + CH, :])
                rp = rp_pool.tile([C, CH, W], bf16)
                nc.sync.dma_start(rp[:], right[:, h0 : h0 + CH, :])

                pts = [
                    ps_pool.tile([128, 4 * BANDW], f32, tag="ps", name=f"ps{t}")
                    for t in range(NTILE)
                ]
                for K in range(NK):
                    wb, sec = K % NWB, K // NWB
                    t, c0 = K // 4, (K % 4) * BANDW
                    x0 = (NWB - 1) * BM - BM * wb  # 288 - 32*wb
                    # cols with w2 = 31 + 32*wb - f < 0 are the w < d zero
                    # triangle: skip computing them (host masks); the band
                    # then holds stale garbage there, never gathered as valid
                    cols = min(BANDW, BM * wb + BM)
                    for s in range(4):
                        hh = 4 * sec + s
                        nc.tensor.matmul(
                            pts[t][32 * s : 32 * s + 32, c0 : c0 + cols],
                            lt[:, hh, BM * wb : BM * wb + BM],
                            rp[:, hh, x0 : x0 + cols],
                            start=True,
                            stop=True,
                            tile_position=(0, 32 * s),
                        )

                last = ci == reps * nchunk - 1
                bb = band_pool.tile([128, PKW], bf16, tag="band")
                for t in range(NTILE):
                    dst = bb[:, 4 * BANDW * t : 4 * BANDW * (t + 1)]
                    if t in (0, 2):
                        nc.vector.tensor_copy(dst, pts[t][:])
                    else:
                        nc.scalar.copy(dst, pts[t][:])
                    if last:
                        # final chunk: store per-tile on the now-idle
                        # SP/HWDGE path so the drain tail is one tile, not
                        # the whole chunk pipeline
                        nc.sync.dma_start(
                            out[c][:, 4 * BANDW * t : 4 * BANDW * (t + 1)], dst
                        )

                if not last:
                    # Pool/SWDGE: keeps the store off both the SP queue
                    # (would serialize next chunk's loads behind this chunk's
                    # eviction sem wait) and the shared HWDGE
                    nc.gpsimd.dma_start(out[c], bb[:])

    nc.compile()
    return nc


def _get_nc():
    key = H
    if key not in _cache:
        _cache[key] = _build()
    return _cache[key]


def _prep(left_feature, right_feature):
    import ml_dtypes

    lf = np.asarray(left_feature, dtype=np.float32) * np.float32(1.0 / C)
    rf = np.asarray(right_feature, dtype=np.float32)[:, :, :, ::-1]
    lf = lf.astype(ml_dtypes.bfloat16)
    rf = np.ascontiguousarray(rf).astype(ml_dtypes.bfloat16)
    return lf, rf


# f_sel[q, d] = 31 - q + d : band col of diagonal d for in-block col q
_FSEL = (BM - 1 - np.arange(BM)[:, None] + np.arange(D)[None, :])[
    None, None, :, None, None, :
]
# w >= d validity mask; w < d entries were never computed on device (stale
# PSUM garbage in the band) and are exact zeros in the reference
_WMASK = (np.arange(W)[None, None, :] >= np.arange(D)[:, None, None])


def _extract(band):
    """[nchunk, 128, PKW] bf16 band -> [D, H, W] f32 volume (pure selection)."""
    nchunk = band.shape[0]
    # [c, s, q, sec, wb, f']: p = 32*s + q, col = 80*(10*sec + wb) + f'
    br = np.asarray(band, dtype=np.float32).reshape(nchunk, 4, BM, 2, NWB, BANDW)
    ext = np.take_along_axis(br, np.broadcast_to(
        _FSEL, (nchunk, 4, BM, 2, NWB, D)), axis=5)
    # h = 8c + 4*sec + s, w = 32*wb + q, d
    # [c, s, q, sec, wb, d] -> [d, c, sec, s, wb, q] -> [D, H, W]
    vol = ext.transpose(5, 0, 3, 1, 4, 2).reshape(D, H, W)
    return np.where(_WMASK, vol, np.float32(0.0))


def kernel(left_feature, right_feature, disp):
    from concourse.bass_utils import run_bass_kernel_spmd

    assert int(disp) == D, f"kernel hardcoded for disp={D}, got {disp}"
    lf, rf = _prep(left_feature, right_feature)
    assert lf.shape == (B, C, H, W), lf.shape

    nc = _get_nc()
    in_maps = [{"left": lf[b], "right": rf[b]} for b in range(B)]
    res = run_bass_kernel_spmd(nc, in_maps, list(range(B)))

    vol = np.empty((B, D, H, W), dtype=np.float32)
    for b in range(B):
        vol[b] = _extract(np.asarray(res.results[b]["out"]))
    return vol


# revision 19
# speedup vs baseline: 1.0028x; 1.0028x over previous
"""Cost-volume kernel for Trainium2 (Bass/Tile), SPMD over 8 NeuronCores.

volume[b, d, h, w] = mean_c left[b,c,h,w] * right[b,c,h,w-d],  0 for w < d.

Per core (one batch image b), per 8-row chunk (CH=8, 20 chunks):
  - Host pre-reverses right along w (rp[c,x] = R[c, 319-x]) and pre-scales
    left by 1/64 (the channel mean, exact power of two), both cast to bf16.
  - TensorE: per (hh, wb) one [32, <=80] matmul over w-blocks of BM=32
      G[q, f] = sum_c L[c, 32*wb+q] * rp[c, 288-32*wb+f]
             = vol_unscaled[d = q + f - 31, h, w = 32*wb + q]
    so each PSUM row is d-contiguous.  Columns with w2 = 31+32*wb-f < 0
    (the w < d zero triangle) are skipped (cols = min(80, 32*wb+32)); the
    host masks them.  Four hh's stack at 32-row offsets (matmul
    tile_position col in {0,32,64,96}) into [128, 320] PSUM tiles, 4
    col-regions per tile, 5 tiles per chunk.
  - DVE (tiles 0,2) / ACT (tiles 1,3,4) evict PSUM -> SBUF band
    [128, 1600] with the f32 -> bf16 cast.
  - ONE contiguous store DMA per chunk: band -> DRAM out[c] (3200B runs,
    full DMA bandwidth; no on-chip reshuffle at all).  The final chunk
    stores per-tile on SP/HWDGE so the drain tail is one tile deep.
Host: gather the 48 diagonals f = 31 - q + d from the band (pure
selection, numpy take_along_axis), mask w < d to exact 0, reshape to
[D, H, W] f32.

Why this shape (TimelineSim cost model, which tracked the measured
baseline within 4%):
  - DMA_ENGINES is the bottleneck resource: busy = loads 36.4us (13.1MB
    bf16 inputs at 360GB/s; bf16 is required - fp8 inputs put ~3.6% rms on
    the output vs the 2e-2 gate) + stores 22.8us.  The 80-wide band
    regions store 1.67x the output bytes but at the full contiguous rate;
    runs under 512B are charged 2x, so trimming the band below full rows
    always loses.  BM=32 balances store overhead ((BM+48)/48) against PE
    column time ((BM+48)/BM): PE ~50us < DMA ~59us.
  - A diagonal-extracting store would need a sub-row partition step on the
    DMA *read* side, which the BIR verifier rejects ("illegal partition
    step" - writes only); hence store-the-band + host selection.
  - Loads go on SP/HWDGE, the band store on Pool/SWDGE: a shared queue
    would serialize chunk c+1's loads behind chunk c's eviction semaphore
    (the store must wait for evictions), collapsing the pipeline.
"""

import sys

sys.path.insert(0, "/opt/trn_rl_repo")

import numpy as np

import concourse.tile as tile
from concourse import bacc, mybir

B, C, H, W, D = 8, 64, 160, 320, 48
MARGIN = 48
BM = 32                      # w-block size
BANDW = BM + MARGIN - 1      # 79 band cols per region (f = 31-q+d <= 78)
NWB = W // BM                # 10 blocks
CH = 8                       # h rows per chunk
NK = 2 * NWB                 # 20 col-regions per chunk (2 sections x 10 wb)
PKW = NK * BANDW             # 1600 band cols
NTILE = 5                    # PSUM tiles per chunk (4 regions each)

_cache = {}


def _build(h_count=H, reps=1):
    bf16 = mybir.dt.bfloat16
    f32 = mybir.dt.float32
    assert h_count % CH == 0
    nchunk = h_count // CH

    nc = bacc.Bacc("TRN2", target_bir_lowering=False, debug=False)
    left = nc.dram_tensor("left", [C, h_count, W], bf16, kind="ExternalInput")
    right = nc.dram_tensor("right", [C, h_count, W], bf16, kind="ExternalInput")
    if reps != 1:
        # unused; forces a distinct HLO per reps so the jit/NEFF caches
        # cannot alias timing builds of different rep counts
        nc.dram_tensor("rep_tag", [1, 8 * reps], mybir.dt.float32,
                       kind="ExternalInput")
    out = nc.dram_tensor("out", [nchunk, 128, PKW], bf16, kind="ExternalOutput")

    with tile.TileContext(nc) as tc:
        with (
            tc.tile_pool(name="lt", bufs=6) as lt_pool,
            tc.tile_pool(name="rp", bufs=6) as rp_pool,
            tc.tile_pool(name="ps", bufs=8, space="PSUM") as ps_pool,
            tc.tile_pool(name="band", bufs=4) as band_pool,
        ):
            for ci in range(reps * nchunk):
                c = ci % nchunk
                h0 = c * CH
                lt = lt_pool.tile([C, CH, W], bf16)
                nc.sync.dma_start(lt[:], left[:, h0 : h0 + CH, :])
                rp = rp_pool.tile([C, CH, W], bf16)
                nc.sync.dma_start(rp[:], right[:, h0 : h0 + CH, :])

                pts = [
                    ps_pool.tile([128, 4 * BANDW], f32, tag="ps", name=f"ps{t}")
                    for t in range(NTILE)
                ]
                for K in range(NK):
                    wb, sec = K % NWB, K // NWB
                    t, c0 = K // 4, (K % 4) * BANDW
                    x0 = (NWB - 1) * BM - BM * wb  # 288 - 32*wb
                    # cols with w2 = 31 + 32*wb - f < 0 are the w < d zero
                    # triangle: skip computing them (host masks); the band
                    # then holds stale garbage there, never gathered as valid
                    cols = min(BANDW, BM * wb + BM)
                    for s in range(4):
                        hh = 4 * sec + s
                        nc.tensor.matmul(
                            pts[t][32 * s : 32 * s + 32, c0 : c0 + cols],
                            lt[:, hh, BM * wb : BM * wb + BM],
                            rp[:, hh, x0 : x0 + cols],
                            start=True,
                            stop=True,
                            tile_position=(0, 32 * s),
                        )

                last = ci == reps * nchunk - 1
                bb = band_pool.tile([128, PKW], bf16, tag="band")
                for t in range(NTILE):
                    dst = bb[:, 4 * BANDW * t : 4 * BANDW * (t + 1)]
                    if t in (0, 2):
                        nc.vector.tensor_copy(dst, pts[t][:])
                    else:
                        nc.scalar.copy(dst, pts[t][:])
                    if last:
                        # final chunk: store per-tile on the now-idle
                        # SP/HWDGE path so the drain tail is one tile, not
                        # the whole chunk pipeline
                        nc.sync.dma_start(
                            out[c][:, 4 * BANDW * t : 4 * BANDW * (t + 1)], dst
                        )

                if not last:
                    # Pool/SWDGE: keeps the store off both the SP queue
                    # (would serialize next chunk's loads behind this chunk's
                    # eviction sem wait) and the shared HWDGE
                    nc.gpsimd.dma_start(out[c], bb[:])

    nc.compile()
    return nc


def _get_nc():
    key = H
    if key not in _cache:
        _cache[key] = _build()
    return _cache[key]


def _prep(left_feature, right_feature):
    import ml_dtypes

    lf = np.asarray(left_feature, dtype=np.float32) * np.float32(1.0 / C)
    rf = np.asarray(right_feature, dtype=np.float32)[:, :, :, ::-1]
    lf = lf.astype(ml_dtypes.bfloat16)
    rf = np.ascontiguousarray(rf).astype(ml_dtypes.bfloat16)
    return lf, rf


# f_sel[q, d] = 31 - q + d : band col of diagonal d for in-block col q
_FSEL = (BM - 1 - np.arange(BM)[:, None] + np.arange(D)[None, :])[
    None, None, :, None, None, :
]
# w >= d validity mask; w < d entries were never computed on device (stale
# PSUM garbage in the band) and are exact zeros in the reference
_WMASK = (np.arange(W)[None, None, :] >= np.arange(D)[:, None, None])


def _extract(band):
    """[nchunk, 128, PKW] bf16 band -> [D, H, W] f32 volume (pure selection)."""
    nchunk = band.shape[0]
    # [c, s, q, sec, wb, f']: p = 32*s + q, col = 80*(10*sec + wb) + f'
    br = np.asarray(band, dtype=np.float32).reshape(nchunk, 4, BM, 2, NWB, BANDW)
    ext = np.take_along_axis(br, np.broadcast_to(
        _FSEL, (nchunk, 4, BM, 2, NWB, D)), axis=5)
    # h = 8c + 4*sec + s, w = 32*wb + q, d
    # [c, s, q, sec, wb, d] -> [d, c, sec, s, wb, q] -> [D, H, W]
    vol = ext.transpose(5, 0, 3, 1, 4, 2).reshape(D, H, W)
    return np.where(_WMASK, vol, np.float32(0.0))


def kernel(left_feature, right_feature, disp):
    from concourse.bass_utils import run_bass_kernel_spmd

    assert int(disp) == D, f"kernel hardcoded for disp={D}, got {disp}"
    lf, rf = _prep(left_feature, right_feature)
    assert lf.shape == (B, C, H, W), lf.shape

    nc = _get_nc()
    in_maps = [{"left": lf[b], "right": rf[b]} for b in range(B)]
    res = run_bass_kernel_spmd(nc, in_maps, list(range(B)))

    vol = np.empty((B, D, H, W), dtype=np.float32)
    for b in range(B):
        vol[b] = _extract(np.asarray(res.results[b]["out"]))
    return vol
